# revision 20
# baseline (speedup 1.0000x reference)
"""Group-wise correlation cost volume (build_gwc_volume) on 8 trn2 cores.

volume[b,g,d,h,w] = sum_c ref[b,g,c,h,w] * tgt[b,g,c,h,w-d]  (0 where w<d)

Sharding: 16 (b,g) pairs across 8 cores, 2 pairs per core. Each pair is a
contiguous 64-channel slice of the inputs and a contiguous [D,H,W] slab of
the output.

Per (b,g,h) the volume rows are diagonals of the Gram matrix
G[w',w] = sum_c tgt[c,w'] * ref[c,w].  Only the band d = w - w' in [0,48)
is needed, so the Gram is computed as 8 column-piece matmuls (M=32,
stationary T[:, 32k:32k+32]), each with an 80-wide moving window
R[:, BASE_k : BASE_k+80) written at a fixed offset of a PSUM bank.
Row p of the result holds G[p, BASE_k + x] — the band sits in a fixed
80-wide window per row.  The two (b,g) pairs sit on PE row halves and the
4 column pieces on PE column groups, so all 16 matmuls per h share the
128x128 array; both pairs' bands pack into ONE PSUM bank (2*160=320 f32
<= 512) so a single [128, 320] copy per h drains PSUM — the copy's fixed
PSUM-access latency amortizes over both pairs.

The kernel is DMA-bound (inputs read once + band tiles written once), so
everything crosses HBM as bf16: inputs are cast on the host, the PSUM f32
band is cast to bf16 on the PSUM->SBUF copy. The 2e-2 rel-err budget
dwarfs the ~4e-3 bf16 error.

Three h-rows pack into each per-pair PSUM bank (3*160 = 480 f32 <= 512),
so one [128, 480] copy drains 3 h-rows — amortizing the ~170ns fixed
PSUM-read latency that otherwise made the copy engines the pipeline
choke point. Banks stay per-pair: writes to one bank must come from one
PE row-half (two row-tiles draining the same bank faults the HW).

Diagonal (shear) extraction at 1-partition granularity is not expressible
in any engine's access patterns, so the 80-wide band tiles are DMAed out
and the diagonals are gathered on the host during unsharding.
"""

import sys

if "/opt/trn_rl_repo" not in sys.path:
    sys.path.insert(0, "/opt/trn_rl_repo")

import ml_dtypes
import numpy as np

import concourse.bacc as bacc
import concourse.tile as tile
from concourse import mybir
from concourse.bass_utils import run_bass_kernel_spmd

F32 = mybir.dt.float32
BF16 = mybir.dt.bfloat16
NP_BF16 = ml_dtypes.bfloat16

B, C, H, W = 2, 512, 128, 256
G, CG, D = 8, 64, 48
N_CORES = 8
PAIRS = 2  # (b,g) pairs per core
HC = 32  # h rows per chunk
PW = 80  # piece window width (32 + 47 + 1)

# piece k covers w' in [32k, 32k+32); its moving window starts at
# BASE[k] = min(32k, W - PW) so every piece is a full 80 columns.
BASE = [min(32 * k, W - PW) for k in range(8)]

_cached = {}


def _build_module():
    nc = bacc.Bacc("TRN2", target_bir_lowering=False, debug=False, num_devices=N_CORES)
    ref = nc.dram_tensor("ref", [PAIRS, CG, H, W], BF16, kind="ExternalInput")
    tgt = nc.dram_tensor("tgt", [PAIRS, CG, H, W], BF16, kind="ExternalInput")
    # band tiles, layout [pair, w'-row, h, x]: cols 0:80 pieces 0-3
    # (w' in [0,128)), cols 80:160 pieces 4-7 (w' in [128,256))
    out_bt = nc.dram_tensor(
        "out_bt", [PAIRS, 128, H, 2 * PW], BF16, kind="ExternalOutput"
    )

    ref_p = ref.rearrange("pr c h w -> (pr c) h w")
    tgt_p = tgt.rearrange("pr c h w -> (pr c) h w")

    with tile.TileContext(nc) as tc:
        with (
            tc.tile_pool(name="rtp", bufs=4) as rt_pool,
            tc.tile_pool(name="ttp", bufs=4) as tt_pool,
            tc.tile_pool(name="stage", bufs=2) as stage_pool,
            tc.tile_pool(name="psum", bufs=4, space="PSUM") as psum,
        ):
            # HAM warm-up: ~4.3us of back-to-back fat matmuls on scratch
            # data while the first input DMAs stream (PE is idle then
            # anyway). Sustained PE activity >3.4us flips the clock gate
            # to 8/8 (1.2 -> 2.4 GHz) for the rest of the kernel; without
            # it every matmul in this kernel measures cold (~222ns for
            # N=80 vs ~130 warm).
            wsrc = stage_pool.tile([128, 640], BF16, tag="warm", name="warm_src")
            nc.vector.memzero(wsrc[:])
            wbank = psum.tile([128, 512], F32, tag="bk0", name="warm_bank")
            for i in range(10):
                nc.tensor.matmul(
                    wbank[:, :], wsrc[:, 0:128], wsrc[:, 128:640]
                )

            # prologue: all 8 input DMAs issue upfront on sync. They fill
            # exactly the 8 HWDGE completion-sem lanes with no wrap, so no
            # input DMA ever chains behind a compute-gated transfer (the
            # 8-lane round-robin was silently serializing the pipeline).
            rts, tts = [], []
            for ch in range(H // HC):
                h0 = ch * HC
                rt = rt_pool.tile([128, HC, W], BF16, tag="rt", name=f"rt_{ch}")
                tt = tt_pool.tile([128, HC, W], BF16, tag="tt", name=f"tt_{ch}")
                nc.sync.dma_start(rt[:], ref_p[:, h0 : h0 + HC, :])
                nc.sync.dma_start(tt[:], tgt_p[:, h0 : h0 + HC, :])
                rts.append(rt)
                tts.append(tt)

            for ch in range(H // HC):
                h0 = ch * HC
                rt = rts[ch]
                tt = tts[ch]
                stages = []
                for pr in range(PAIRS):
                    st = stage_pool.tile(
                        [128, HC, 2 * PW], BF16, tag=f"st{pr}", name=f"st{pr}_{ch}"
                    )
                    stages.append(st)
                for i, hl0 in enumerate(range(0, HC, 3)):
                    hn = min(3, HC - hl0)  # h-rows packed in this bank
                    for pr in range(PAIRS):
                        p0 = pr * CG
                        bank = psum.tile(
                            [128, 3, 2 * PW],
                            F32,
                            tag=f"bk{pr}",
                            name=f"bk{pr}_{ch}_{hl0}",
                        )
                        for hj in range(hn):
                            hl = hl0 + hj
                            for k in range(8):
                                c0 = PW * (k // 4)
                                m0 = 32 * (k % 4)
                                nc.tensor.matmul(
                                    bank[m0 : m0 + 32, hj, c0 : c0 + PW],
                                    tt[p0 : p0 + CG, hl, 32 * k : 32 * k + 32],
                                    rt[p0 : p0 + CG, hl, BASE[k] : BASE[k] + PW],
                                    tile_position=(p0, m0),
                                )
                        # all copies on DVE, all out-DMA on ACT, all in-DMA
                        # on sync: one instruction type per strict-FIFO
                        # queue, so a sem-waiting head never blocks an
                        # unrelated transfer behind it
                        nc.vector.tensor_copy(
                            stages[pr][:, hl0 : hl0 + hn, :], bank[:, :hn, :]
                        )
                for pr in range(PAIRS):
                    # SWDGE: out-DMAs complete on the separate DMASW sem
                    # lanes, fully decoupled from the input DMA window
                    nc.gpsimd.dma_start(
                        out_bt[pr, :, h0 : h0 + HC, :], stages[pr][:]
                    )

    nc.compile()
    return nc


def _get_module():
    if "nc" not in _cached:
        _cached["nc"] = _build_module()
    return _cached["nc"]


def _host_extract(bt):
    """Gather band diagonals into the full volume.

    bt: [16, 128, H, 160] f32.  Row p holds G[w', w = BASE[k] + x] at col
    80*(k//4) + x where k = w'//32 indexes the piece (pieces 0-3 at cols
    0:80 for w' = row, pieces 4-7 at cols 80:160 for w' = row + 128).
    vol[d,h,w] = G[w-d, w] -> row (w-d) % 128, col from the piece table.
    """
    d = np.arange(D)[:, None]
    w = np.arange(W)[None, :]
    wp = w - d  # [D, W] source w' (negative -> zero region)
    valid = wp >= 0
    wpc = np.clip(wp, 0, None)
    k = wpc // 32
    base = np.minimum(32 * k, W - PW)
    col = PW * (k // 4) + (w - base)
    row = wpc % 128

    vol = np.zeros((B * G, D, H, W), np.float32)
    for pair in range(B * G):
        t = bt[pair].transpose(1, 0, 2)  # [h, row, col]
        r = t[:, row, col]  # [H, D, W]
        r *= valid[None]
        vol[pair] = r.transpose(1, 0, 2)
    return vol.reshape(B, G, D, H, W)


def kernel(refimg_fea, targetimg_fea, num_groups, maxdisp):
    assert int(num_groups) == G and int(maxdisp) == D
    ref = np.asarray(refimg_fea, dtype=np.float32).astype(NP_BF16)
    tgt = np.asarray(targetimg_fea, dtype=np.float32).astype(NP_BF16)
    assert ref.shape == (B, C, H, W)

    rp = np.ascontiguousarray(ref.reshape(B * G, CG, H, W))
    tp = np.ascontiguousarray(tgt.reshape(B * G, CG, H, W))
    in_maps = [
        {"ref": rp[2 * k : 2 * k + 2], "tgt": tp[2 * k : 2 * k + 2]}
        for k in range(N_CORES)
    ]

    nc = _get_module()
    res = run_bass_kernel_spmd(nc, in_maps, core_ids=list(range(N_CORES)))

    bt = np.concatenate(
        [np.asarray(r["out_bt"]).astype(np.float32) for r in res.results], axis=0
    )
    return _host_extract(bt)


# revision 23
# speedup vs baseline: 1.0253x; 1.0253x over previous
"""Group-wise correlation cost volume (build_gwc_volume) on 8 trn2 cores.

volume[b,g,d,h,w] = sum_c ref[b,g,c,h,w] * tgt[b,g,c,h,w-d]  (0 where w<d)

Sharding: 16 (b,g) pairs across 8 cores, 2 pairs per core. Each pair is a
contiguous 64-channel slice of the inputs and a contiguous [D,H,W] slab of
the output.

Per (b,g,h) the volume rows are diagonals of the Gram matrix
G[w',w] = sum_c tgt[c,w'] * ref[c,w].  Only the band d = w - w' in [0,48)
is needed, so the Gram is computed as 8 column-piece matmuls (M=32,
stationary T[:, 32k:32k+32]), each with an 80-wide moving window
R[:, BASE_k : BASE_k+80) written at a fixed offset of a PSUM bank.
Row p of the result holds G[p, BASE_k + x] — the band sits in a fixed
80-wide window per row.  The two (b,g) pairs sit on PE row halves and the
4 column pieces on PE column groups, so all 16 matmuls per h share the
128x128 array; both pairs' bands pack into ONE PSUM bank (2*160=320 f32
<= 512) so a single [128, 320] copy per h drains PSUM — the copy's fixed
PSUM-access latency amortizes over both pairs.

The kernel is DMA-bound (inputs read once + band tiles written once), so
everything crosses HBM as bf16: inputs are cast on the host, the PSUM f32
band is cast to bf16 on the PSUM->SBUF copy. The 2e-2 rel-err budget
dwarfs the ~4e-3 bf16 error.

Three h-rows pack into each per-pair PSUM bank (3*160 = 480 f32 <= 512),
so one [128, 480] copy drains 3 h-rows — amortizing the ~170ns fixed
PSUM-read latency that otherwise made the copy engines the pipeline
choke point. Banks stay per-pair: writes to one bank must come from one
PE row-half (two row-tiles draining the same bank faults the HW).

Diagonal (shear) extraction at 1-partition granularity is not expressible
in any engine's access patterns, so the 80-wide band tiles are DMAed out
and the diagonals are gathered on the host during unsharding.
"""

import sys

if "/opt/trn_rl_repo" not in sys.path:
    sys.path.insert(0, "/opt/trn_rl_repo")

import ml_dtypes
import numpy as np

import concourse.bacc as bacc
import concourse.tile as tile
from concourse import mybir
from concourse.bass_utils import run_bass_kernel_spmd

F32 = mybir.dt.float32
BF16 = mybir.dt.bfloat16
NP_BF16 = ml_dtypes.bfloat16

B, C, H, W = 2, 512, 128, 256
G, CG, D = 8, 64, 48
N_CORES = 8
PAIRS = 2  # (b,g) pairs per core
HC = 32  # h rows per chunk
PW = 80  # piece window width (32 + 47 + 1)

# piece k covers w' in [32k, 32k+32); its moving window starts at
# BASE[k] = min(32k, W - PW) so every piece is a full 80 columns.
BASE = [min(32 * k, W - PW) for k in range(8)]

_cached = {}


def _build_module():
    nc = bacc.Bacc("TRN2", target_bir_lowering=False, debug=False, num_devices=N_CORES)
    ref = nc.dram_tensor("ref", [PAIRS, CG, H, W], BF16, kind="ExternalInput")
    tgt = nc.dram_tensor("tgt", [PAIRS, CG, H, W], BF16, kind="ExternalInput")
    # band tiles, layout [pair, w'-row, h, x]: cols 0:80 pieces 0-3
    # (w' in [0,128)), cols 80:160 pieces 4-7 (w' in [128,256))
    out_bt = nc.dram_tensor(
        "out_bt", [PAIRS, 128, H, 2 * PW], BF16, kind="ExternalOutput"
    )

    ref_p = ref.rearrange("pr c h w -> (pr c) h w")
    tgt_p = tgt.rearrange("pr c h w -> (pr c) h w")

    with tile.TileContext(nc) as tc:
        with (
            tc.tile_pool(name="rtp", bufs=4) as rt_pool,
            tc.tile_pool(name="ttp", bufs=4) as tt_pool,
            tc.tile_pool(name="stage", bufs=2) as stage_pool,
            tc.tile_pool(name="psum", bufs=4, space="PSUM") as psum,
        ):
            # HAM warm-up: ~4.3us of back-to-back fat matmuls on scratch
            # data while the first input DMAs stream (PE is idle then
            # anyway). Sustained PE activity >3.4us flips the clock gate
            # to 8/8 (1.2 -> 2.4 GHz) for the rest of the kernel; without
            # it every matmul in this kernel measures cold (~222ns for
            # N=80 vs ~130 warm).
            wsrc = stage_pool.tile([128, 640], BF16, tag="warm", name="warm_src")
            nc.vector.memzero(wsrc[:])
            wbank = psum.tile([128, 512], F32, tag="bk0", name="warm_bank")
            for i in range(16):
                nc.tensor.matmul(
                    wbank[:, :], wsrc[:, 0:128], wsrc[:, 128:640]
                )

            # prologue: all 8 input DMAs issue upfront on sync. They fill
            # exactly the 8 HWDGE completion-sem lanes with no wrap, so no
            # input DMA ever chains behind a compute-gated transfer (the
            # 8-lane round-robin was silently serializing the pipeline).
            rts, tts = [], []
            for ch in range(H // HC):
                h0 = ch * HC
                rt = rt_pool.tile([128, HC, W], BF16, tag="rt", name=f"rt_{ch}")
                tt = tt_pool.tile([128, HC, W], BF16, tag="tt", name=f"tt_{ch}")
                nc.sync.dma_start(rt[:], ref_p[:, h0 : h0 + HC, :])
                nc.sync.dma_start(tt[:], tgt_p[:, h0 : h0 + HC, :])
                rts.append(rt)
                tts.append(tt)

            for ch in range(H // HC):
                h0 = ch * HC
                rt = rts[ch]
                tt = tts[ch]
                stages = []
                for pr in range(PAIRS):
                    st = stage_pool.tile(
                        [128, HC, 2 * PW], BF16, tag=f"st{pr}", name=f"st{pr}_{ch}"
                    )
                    stages.append(st)
                for i, hl0 in enumerate(range(0, HC, 3)):
                    hn = min(3, HC - hl0)  # h-rows packed in this bank
                    for pr in range(PAIRS):
                        p0 = pr * CG
                        bank = psum.tile(
                            [128, 3, 2 * PW],
                            F32,
                            tag=f"bk{pr}",
                            name=f"bk{pr}_{ch}_{hl0}",
                        )
                        for hj in range(hn):
                            hl = hl0 + hj
                            for k in range(8):
                                c0 = PW * (k // 4)
                                m0 = 32 * (k % 4)
                                nc.tensor.matmul(
                                    bank[m0 : m0 + 32, hj, c0 : c0 + PW],
                                    tt[p0 : p0 + CG, hl, 32 * k : 32 * k + 32],
                                    rt[p0 : p0 + CG, hl, BASE[k] : BASE[k] + PW],
                                    tile_position=(p0, m0),
                                )
                        # copies alternate DVE/ACT: the ~500ns PSUM-drain
                        # copies are half the kernel's compute time, and
                        # ACT is otherwise idle
                        if (i + pr) % 2 == 0:
                            nc.vector.tensor_copy(
                                stages[pr][:, hl0 : hl0 + hn, :], bank[:, :hn, :]
                            )
                        else:
                            nc.scalar.copy(
                                stages[pr][:, hl0 : hl0 + hn, :], bank[:, :hn, :]
                            )
                for pr in range(PAIRS):
                    # on sync's HWDGE ring, queued after the 8 prologue
                    # input DMAs: all lane predecessors are early input
                    # transfers, so no out-DMA chains behind compute
                    nc.sync.dma_start(
                        out_bt[pr, :, h0 : h0 + HC, :], stages[pr][:]
                    )

    nc.compile()
    return nc


def _get_module():
    if "nc" not in _cached:
        _cached["nc"] = _build_module()
    return _cached["nc"]


def _host_extract(bt):
    """Gather band diagonals into the full volume.

    bt: [16, 128, H, 160] f32.  Row p holds G[w', w = BASE[k] + x] at col
    80*(k//4) + x where k = w'//32 indexes the piece (pieces 0-3 at cols
    0:80 for w' = row, pieces 4-7 at cols 80:160 for w' = row + 128).
    vol[d,h,w] = G[w-d, w] -> row (w-d) % 128, col from the piece table.
    """
    d = np.arange(D)[:, None]
    w = np.arange(W)[None, :]
    wp = w - d  # [D, W] source w' (negative -> zero region)
    valid = wp >= 0
    wpc = np.clip(wp, 0, None)
    k = wpc // 32
    base = np.minimum(32 * k, W - PW)
    col = PW * (k // 4) + (w - base)
    row = wpc % 128

    vol = np.zeros((B * G, D, H, W), np.float32)
    for pair in range(B * G):
        t = bt[pair].transpose(1, 0, 2)  # [h, row, col]
        r = t[:, row, col]  # [H, D, W]
        r *= valid[None]
        vol[pair] = r.transpose(1, 0, 2)
    return vol.reshape(B, G, D, H, W)


def kernel(refimg_fea, targetimg_fea, num_groups, maxdisp):
    assert int(num_groups) == G and int(maxdisp) == D
    ref = np.asarray(refimg_fea, dtype=np.float32).astype(NP_BF16)
    tgt = np.asarray(targetimg_fea, dtype=np.float32).astype(NP_BF16)
    assert ref.shape == (B, C, H, W)

    rp = np.ascontiguousarray(ref.reshape(B * G, CG, H, W))
    tp = np.ascontiguousarray(tgt.reshape(B * G, CG, H, W))
    in_maps = [
        {"ref": rp[2 * k : 2 * k + 2], "tgt": tp[2 * k : 2 * k + 2]}
        for k in range(N_CORES)
    ]

    nc = _get_module()
    res = run_bass_kernel_spmd(nc, in_maps, core_ids=list(range(N_CORES)))

    bt = np.concatenate(
        [np.asarray(r["out_bt"]).astype(np.float32) for r in res.results], axis=0
    )
    return _host_extract(bt)


# revision 26
# speedup vs baseline: 1.1642x; 1.1355x over previous
"""Group-wise correlation cost volume (build_gwc_volume) on 8 trn2 cores.

volume[b,g,d,h,w] = sum_c ref[b,g,c,h,w] * tgt[b,g,c,h,w-d]  (0 where w<d)

Sharding: 16 (b,g) pairs across 8 cores, 2 pairs per core. Each pair is a
contiguous 64-channel slice of the inputs and a contiguous [D,H,W] slab of
the output.

Per (b,g,h) the volume rows are diagonals of the Gram matrix
G[w',w] = sum_c tgt[c,w'] * ref[c,w].  Only the band d = w - w' in [0,48)
is needed, so the Gram is computed as 8 column-piece matmuls (M=32,
stationary T[:, 32k:32k+32]), each with an 80-wide moving window
R[:, BASE_k : BASE_k+80) written at a fixed offset of a PSUM bank.
Row p of the result holds G[p, BASE_k + x] — the band sits in a fixed
80-wide window per row.  The two (b,g) pairs sit on PE row halves and the
4 column pieces on PE column groups, so all 16 matmuls per h share the
128x128 array.

The kernel is DMA-bound (inputs read once + band tiles written once), so
everything crosses HBM as bf16: inputs are cast on the host, the PSUM f32
band is cast to bf16 on the PSUM->SBUF copy. The 2e-2 rel-err budget
dwarfs the ~4e-3 bf16 error.

Three h-rows pack into each per-pair PSUM bank (3*160 = 480 f32 <= 512),
so one [128, 480] copy drains 3 h-rows — amortizing the ~170ns fixed
PSUM-read latency that otherwise made the copy engines the pipeline
choke point. Banks stay per-pair: writes to one bank must come from one
PE row-half (two row-tiles draining the same bank faults the HW).

Diagonal (shear) extraction at 1-partition granularity is not expressible
in any engine's access patterns, so the 80-wide band tiles are DMAed out
and the diagonals are gathered on the host during unsharding.
"""

import sys

if "/opt/trn_rl_repo" not in sys.path:
    sys.path.insert(0, "/opt/trn_rl_repo")

import ml_dtypes
import numpy as np

import concourse.bacc as bacc
import concourse.tile as tile
from concourse import mybir
from concourse.bass_utils import run_bass_kernel_spmd

F32 = mybir.dt.float32
BF16 = mybir.dt.bfloat16
NP_BF16 = ml_dtypes.bfloat16

B, C, H, W = 2, 512, 128, 256
G, CG, D = 8, 64, 48
N_CORES = 8
PAIRS = 2  # (b,g) pairs per core
HC = 16  # h rows per chunk
PW = 80  # piece window width (32 + 47 + 1)

# piece k covers w' in [32k, 32k+32); its moving window starts at
# BASE[k] = min(32k, W - PW) so every piece is a full 80 columns.
BASE = [min(32 * k, W - PW) for k in range(8)]

_cached = {}


def _build_module():
    nc = bacc.Bacc("TRN2", target_bir_lowering=False, debug=False, num_devices=N_CORES)
    ref = nc.dram_tensor("ref", [PAIRS, CG, H, W], BF16, kind="ExternalInput")
    tgt = nc.dram_tensor("tgt", [PAIRS, CG, H, W], BF16, kind="ExternalInput")
    # band tiles, layout [pair, w'-row, h, x]: cols 0:80 pieces 0-3
    # (w' in [0,128)), cols 80:160 pieces 4-7 (w' in [128,256))
    out_bt = nc.dram_tensor(
        "out_bt", [PAIRS, 128, H, 2 * PW], BF16, kind="ExternalOutput"
    )

    ref_p = ref.rearrange("pr c h w -> (pr c) h w")
    tgt_p = tgt.rearrange("pr c h w -> (pr c) h w")

    with tile.TileContext(nc) as tc:
        with (
            tc.tile_pool(name="ins", bufs=3) as ins,
            tc.tile_pool(name="stage", bufs=3) as stage_pool,
            tc.tile_pool(name="psum", bufs=4, space="PSUM") as psum,
        ):
            # HAM warm-up: ~4.3us of back-to-back fat matmuls on scratch
            # data while the first input DMAs stream (PE is idle then
            # anyway). Sustained PE activity >3.4us flips the clock gate
            # to 8/8 (1.2 -> 2.4 GHz) for the rest of the kernel; without
            # it every matmul in this kernel measures cold (~222ns for
            # N=80 vs ~130 warm).
            wsrc = stage_pool.tile([128, 640], BF16, tag="warm", name="warm_src")
            nc.vector.memzero(wsrc[:])
            wbank = psum.tile([128, 512], F32, tag="bk0", name="warm_bank")
            for i in range(10):
                nc.tensor.matmul(
                    wbank[:, :], wsrc[:, 0:128], wsrc[:, 128:640]
                )

            for ch in range(H // HC):
                h0 = ch * HC
                rt = ins.tile([128, HC, W], BF16, tag="rt")
                tt = ins.tile([128, HC, W], BF16, tag="tt")
                nc.sync.dma_start(rt[:], ref_p[:, h0 : h0 + HC, :])
                nc.sync.dma_start(tt[:], tgt_p[:, h0 : h0 + HC, :])
                stages = []
                for pr in range(PAIRS):
                    st = stage_pool.tile(
                        [128, HC, 2 * PW], BF16, tag=f"st{pr}", name=f"st{pr}_{ch}"
                    )
                    stages.append(st)
                for i, hl0 in enumerate(range(0, HC, 3)):
                    hn = min(3, HC - hl0)  # h-rows packed in this bank
                    for pr in range(PAIRS):
                        p0 = pr * CG
                        bank = psum.tile(
                            [128, 3, 2 * PW],
                            F32,
                            tag=f"bk{pr}",
                            name=f"bk{pr}_{ch}_{hl0}",
                        )
                        for hj in range(hn):
                            hl = hl0 + hj
                            for k in range(8):
                                c0 = PW * (k // 4)
                                m0 = 32 * (k % 4)
                                nc.tensor.matmul(
                                    bank[m0 : m0 + 32, hj, c0 : c0 + PW],
                                    tt[p0 : p0 + CG, hl, 32 * k : 32 * k + 32],
                                    rt[p0 : p0 + CG, hl, BASE[k] : BASE[k] + PW],
                                    tile_position=(p0, m0),
                                )
                        # all copies on DVE, out-DMA on ACT, in-DMA on
                        # sync: one instruction type per strict-FIFO queue
                        nc.vector.tensor_copy(
                            stages[pr][:, hl0 : hl0 + hn, :], bank[:, :hn, :]
                        )
                for pr in range(PAIRS):
                    nc.scalar.dma_start(
                        out_bt[pr, :, h0 : h0 + HC, :], stages[pr][:]
                    )

    nc.compile()
    return nc


def _get_module():
    if "nc" not in _cached:
        _cached["nc"] = _build_module()
    return _cached["nc"]


def _host_extract(bt):
    """Gather band diagonals into the full volume.

    bt: [16, 128, H, 160] f32.  Row p holds G[w', w = BASE[k] + x] at col
    80*(k//4) + x where k = w'//32 indexes the piece (pieces 0-3 at cols
    0:80 for w' = row, pieces 4-7 at cols 80:160 for w' = row + 128).
    vol[d,h,w] = G[w-d, w] -> row (w-d) % 128, col from the piece table.
    """
    d = np.arange(D)[:, None]
    w = np.arange(W)[None, :]
    wp = w - d  # [D, W] source w' (negative -> zero region)
    valid = wp >= 0
    wpc = np.clip(wp, 0, None)
    k = wpc // 32
    base = np.minimum(32 * k, W - PW)
    col = PW * (k // 4) + (w - base)
    row = wpc % 128

    vol = np.zeros((B * G, D, H, W), np.float32)
    for pair in range(B * G):
        t = bt[pair].transpose(1, 0, 2)  # [h, row, col]
        r = t[:, row, col]  # [H, D, W]
        r *= valid[None]
        vol[pair] = r.transpose(1, 0, 2)
    return vol.reshape(B, G, D, H, W)


def kernel(refimg_fea, targetimg_fea, num_groups, maxdisp):
    assert int(num_groups) == G and int(maxdisp) == D
    ref = np.asarray(refimg_fea, dtype=np.float32).astype(NP_BF16)
    tgt = np.asarray(targetimg_fea, dtype=np.float32).astype(NP_BF16)
    assert ref.shape == (B, C, H, W)

    rp = np.ascontiguousarray(ref.reshape(B * G, CG, H, W))
    tp = np.ascontiguousarray(tgt.reshape(B * G, CG, H, W))
    in_maps = [
        {"ref": rp[2 * k : 2 * k + 2], "tgt": tp[2 * k : 2 * k + 2]}
        for k in range(N_CORES)
    ]

    nc = _get_module()
    res = run_bass_kernel_spmd(nc, in_maps, core_ids=list(range(N_CORES)))

    bt = np.concatenate(
        [np.asarray(r["out_bt"]).astype(np.float32) for r in res.results], axis=0
    )
    return _host_extract(bt)
